# revision 1
# baseline (speedup 1.0000x reference)
"""Trainium2 Bass kernel for a DeepSeek-MLA-style differential-attention layer.

Sharding: tensor-parallel over heads. 16 heads / 8 cores = 2 heads per core.
Each core computes the full low-rank projections (replicated), its 2 heads of
attention, and a partial output projection; the host sums the 8 partials
(the "all-reduce after wo").

Layouts are feature-major ("transposed", [feature, seq]) end to end so every
matmul contraction lands on the partition dimension with no on-device
transposes:
  - scores are computed k-major  sT[kpos, qpos] = kT.T-contraction
  - softmax partition-sums use a ones-vector matmul on the PE
  - attn @ v consumes the k-major exp tiles directly (lhsT = v[kpos, dv])
RoPE pairs are de-interleaved into (real | imag) blocks by permuting weight
rows on the host (dot products are permutation-invariant as long as q and k
use the same permutation).
"""

import math

import numpy as np

import concourse.mybir as mybir
import concourse.tile as tile
from concourse import bacc
from concourse.bass_utils import run_bass_kernel_spmd

F32 = mybir.dt.float32
F32R = mybir.dt.float32r

DIM = 2048
NH = 16
QLR = 768
KVLR = 512
DN = 128
DR = 64
DV = 128
QKH = DN + DR          # 192
H = QKH // 2           # 96
SEQ = 2048
N_CORES = 8
HPC = NH // N_CORES    # heads per core = 2
LAYER_IDX = 3
LAMBDA_INIT = 0.8 - 0.6 * math.exp(-0.3 * LAYER_IDX)
SCALE = QKH ** -0.5
MAX_SCORE = 100.0

KP = 128               # kpos block (partition dim of k-major score tiles)
QP = 512               # qpos block (free dim of score tiles)
NKB = SEQ // KP        # 16 kpos blocks
NQB = SEQ // QP        # 4 qpos blocks

_rope_block_perm = np.concatenate([np.arange(0, DR, 2), np.arange(1, DR, 2)])


SW = SEQ // N_CORES      # per-core seq slice width in launch 1 (256)


def _prep_shared(x, wq_a, wkv_a, freqs_cos, freqs_sin, mask):
    """Host-side layout prep shared by all cores (all cheap reshapes)."""
    x0 = np.ascontiguousarray(x.reshape(SEQ, DIM).astype(np.float32))

    # xT tiled partition-major: [128, DIM//128, SEQ]
    xT = x0.T                                        # [DIM, SEQ]
    xT_t = np.ascontiguousarray(
        xT.reshape(DIM // 128, 128, SEQ).transpose(1, 0, 2))

    # wq_aT: lhsT for q_a projection, [DIM, QLR] tiled -> [128, 16, QLR]
    wq_aT = wq_a.T.astype(np.float32)                # [DIM, QLR]
    wq_aT_t = np.ascontiguousarray(
        wq_aT.reshape(DIM // 128, 128, QLR).transpose(1, 0, 2))

    # wkv_a rows: [0:512 kv | 512:576 k_pe]; de-interleave k_pe rows.
    perm = np.concatenate([np.arange(KVLR), KVLR + _rope_block_perm])
    wkv_aT = wkv_a[perm].T.astype(np.float32)        # [DIM, 576]
    wkv_aT_t = np.ascontiguousarray(
        wkv_aT.reshape(DIM // 128, 128, KVLR + DR).transpose(1, 0, 2))

    # cos/sin transposed and stacked twice: [64, SEQ]
    cosT = freqs_cos.T.astype(np.float32)            # [32, SEQ]
    sinT = freqs_sin.T.astype(np.float32)
    cosT2 = np.ascontiguousarray(np.vstack([cosT, cosT]))
    sinT2 = np.ascontiguousarray(np.vstack([sinT, sinT]))

    ones = np.ones((128, 1), np.float32)

    # Mask block analysis (k-major blocks: [KP kpos, QP qpos]).
    mask = np.asarray(mask, np.float32)
    block_kind = np.zeros((NKB, NQB), np.int8)       # 0=allowed 1=skip 2=partial
    uniq = {}
    block_maskid = -np.ones((NKB, NQB), np.int64)
    emask_list = []
    for n in range(NQB):
        for m in range(NKB):
            blk = mask[n * QP:(n + 1) * QP, m * KP:(m + 1) * KP].T  # [KP, QP]
            if np.all(blk == 0.0):
                block_kind[m, n] = 0
            elif np.all(blk <= -1e8):
                block_kind[m, n] = 1
            else:
                block_kind[m, n] = 2
                e = np.exp(np.maximum(blk, -200.0)).astype(np.float32)
                key = e.tobytes()
                if key not in uniq:
                    uniq[key] = len(emask_list)
                    emask_list.append(e)
                block_maskid[m, n] = uniq[key]
    emask = (np.stack(emask_list) if emask_list
             else np.zeros((1, KP, QP), np.float32))  # [NM, KP, QP]

    xT3 = xT_t                                       # [128, KD, SEQ]
    shared_a = dict(wq_aT=wq_aT_t.reshape(128, -1),
                    wkv_aT=wkv_aT_t.reshape(128, -1))
    x_slices = [np.ascontiguousarray(
        xT3[:, :, c * SW:(c + 1) * SW]).reshape(128, -1)
        for c in range(N_CORES)]
    shared_b = dict(cosT2=cosT2, sinT2=sinT2, ones=ones,
                    emask=np.ascontiguousarray(
                        emask.transpose(1, 0, 2).reshape(KP, -1)))
    return (shared_a, x_slices, shared_b,
            block_kind, block_maskid, len(emask_list))


def _prep_core(core, wq_b, wkv_b, wo, lam):
    """Per-core weight shards (heads 2*core, 2*core+1)."""
    h0, h1 = HPC * core, HPC * core + 1

    # wq_b rows, permuted: [h0 nope | h1 nope | h0r h1r h0i h1i], SCALE folded.
    def q_rows(h):
        base = h * QKH
        nope = np.arange(base, base + DN)
        rope = base + DN + _rope_block_perm
        return nope, rope
    n0, r0 = q_rows(h0)
    n1, r1 = q_rows(h1)
    rows = np.concatenate([n0, n1, r0[:32], r1[:32], r0[32:], r1[32:]])
    wq_bT = (wq_b[rows] * SCALE).T.astype(np.float32)       # [QLR, 384]
    wq_bT_t = np.ascontiguousarray(
        wq_bT.reshape(QLR // 128, 128, HPC * QKH).transpose(1, 0, 2))

    # wkv_b rows: [h0 knope | h0 v | h1 knope | h1 v]
    def kv_rows(h):
        base = h * (DN + DV)
        return np.arange(base, base + DN), np.arange(base + DN, base + DN + DV)
    kn0, v0 = kv_rows(h0)
    kn1, v1 = kv_rows(h1)
    rows = np.concatenate([kn0, v0, kn1, v1])
    wkv_bT = wkv_b[rows].T.astype(np.float32)               # [KVLR, 512]
    wkv_bT_t = np.ascontiguousarray(
        wkv_bT.reshape(KVLR // 128, 128, HPC * (DN + DV)).transpose(1, 0, 2))

    # wo columns for these heads, (1 - LAMBDA_INIT) folded.
    cols = np.concatenate([np.arange(h0 * DV, (h0 + 1) * DV),
                           np.arange(h1 * DV, (h1 + 1) * DV)])
    woT = (wo[:, cols] * (1.0 - LAMBDA_INIT)).T.astype(np.float32)  # [256, DIM]
    woT_t = np.ascontiguousarray(
        woT.reshape(2, 128, DIM).transpose(1, 0, 2))

    return dict(wq_bT=wq_bT_t.reshape(128, -1),
                wkv_bT=wkv_bT_t.reshape(128, -1),
                woT=woT_t.reshape(128, -1),
                lam=np.full((1, 1), lam, np.float32))


def _host_prep(inputs):
    lam = (math.exp(float(np.dot(inputs["lambda_q_nope"],
                                 inputs["lambda_k_nope"])))
           - math.exp(float(np.dot(inputs["lambda_q_rope"],
                                   inputs["lambda_k_rope"])))
           + LAMBDA_INIT)
    (shared_a, x_slices, shared_b, block_kind, block_maskid,
     n_emask) = _prep_shared(
        inputs["x"], inputs["wq_a"], inputs["wkv_a"],
        inputs["freqs_cos"], inputs["freqs_sin"], inputs["mask"])
    in_maps_a = [dict(shared_a, xTs=x_slices[c]) for c in range(N_CORES)]
    in_maps_b = []
    for c in range(N_CORES):
        m = dict(shared_b)
        m.update(_prep_core(c, inputs["wq_b"], inputs["wkv_b"],
                            inputs["wo"], lam))
        in_maps_b.append(m)
    return in_maps_a, in_maps_b, block_kind, block_maskid, n_emask


def _build_a(nc, repeat=1):
    """Launch 1: seq-sharded low-rank projections. Each core's xTs input is
    its own 256-column slice of x^T; outputs are that slice of qa/kv/kpe."""
    KD = DIM // 128
    KQ = QLR // 128
    KV = KVLR // 128
    xTs = nc.dram_tensor("xTs", [128, KD * SW], F32R, kind="ExternalInput") \
        .ap().rearrange("p (k s) -> p k s", k=KD)
    wq_aT = nc.dram_tensor("wq_aT", [128, KD * QLR], F32R,
                           kind="ExternalInput").ap() \
        .rearrange("p (k m) -> p k m", k=KD)
    wkv_aT = nc.dram_tensor("wkv_aT", [128, KD * (KVLR + DR)], F32R,
                            kind="ExternalInput").ap() \
        .rearrange("p (k m) -> p k m", k=KD)
    qa_s = nc.dram_tensor("qa_s", [128, KQ * SW], F32, kind="ExternalOutput") \
        .ap()
    kv_s = nc.dram_tensor("kv_s", [128, KV * SW], F32, kind="ExternalOutput") \
        .ap()
    kpe_s = nc.dram_tensor("kpe_s", [DR, SW], F32, kind="ExternalOutput").ap()

    with tile.TileContext(nc) as tc:
        for _rep in range(repeat):
            with tc.tile_pool(name="pa", bufs=1) as pa, \
                 tc.tile_pool(name="paw", bufs=4) as paw, \
                 tc.tile_pool(name="pas", bufs=4) as pas, \
                 tc.tile_pool(name="psA", bufs=4, space="PSUM") as psA:
                xt = pa.tile([128, KD, SW], F32R, tag="xt")
                nc.sync.dma_start(out=xt, in_=xTs)
                for m in range(KQ + KV + 1):
                    if m < KQ:
                        w_src = wq_aT[:, :, m * 128:(m + 1) * 128]
                        mwid = 128
                    elif m < KQ + KV:
                        mk = m - KQ
                        w_src = wkv_aT[:, :, mk * 128:(mk + 1) * 128]
                        mwid = 128
                    else:
                        w_src = wkv_aT[:, :, KVLR:KVLR + DR]
                        mwid = DR
                    wch = paw.tile([128, KD, 128], F32R, tag="wch")
                    nc.sync.dma_start(out=wch[:, :, :mwid], in_=w_src)
                    ps = psA.tile([128, SW], F32, tag="psA")
                    for k in range(KD):
                        nc.tensor.matmul(ps[:mwid], wch[:, k, :mwid],
                                         xt[:, k, :],
                                         start=(k == 0), stop=(k == KD - 1))
                    stg = pas.tile([128, SW], F32, tag="stg")
                    nc.vector.tensor_copy(stg[:mwid], ps[:mwid])
                    if m < KQ:
                        nc.sync.dma_start(
                            out=qa_s[:, m * SW:(m + 1) * SW], in_=stg)
                    elif m < KQ + KV:
                        mk = m - KQ
                        nc.sync.dma_start(
                            out=kv_s[:, mk * SW:(mk + 1) * SW], in_=stg)
                    else:
                        nc.sync.dma_start(out=kpe_s, in_=stg[:DR])


def _build_be(nc, block_kind, block_maskid, n_emask, repeat=1, until=9):
    NM = max(n_emask, 1)
    KD = DIM // 128          # 16 contraction chunks over DIM
    KQ = QLR // 128          # 6 over QLR
    KV = KVLR // 128         # 4 over KVLR
    MM = mybir.AluOpType
    Exp = mybir.ActivationFunctionType.Exp

    qa_d = nc.dram_tensor("qa_d", [128, KQ * SEQ], F32R,
                          kind="ExternalInput").ap() \
        .rearrange("p (k s) -> p k s", k=KQ)
    kv_d = nc.dram_tensor("kv_d", [128, KV * SEQ], F32R,
                          kind="ExternalInput").ap() \
        .rearrange("p (k s) -> p k s", k=KV)
    kpe_d = nc.dram_tensor("kpe_d", [DR, SEQ], F32R, kind="ExternalInput").ap()
    wq_bT = nc.dram_tensor("wq_bT", [128, KQ * HPC * QKH], F32R,
                           kind="ExternalInput").ap() \
        .rearrange("p (k m) -> p k m", k=KQ)
    wkv_bT = nc.dram_tensor("wkv_bT", [128, KV * HPC * (DN + DV)], F32R,
                            kind="ExternalInput").ap() \
        .rearrange("p (k m) -> p k m", k=KV)
    woT = nc.dram_tensor("woT", [128, 2 * DIM], F32R, kind="ExternalInput").ap() \
        .rearrange("p (k m) -> p k m", k=2)
    cosT2 = nc.dram_tensor("cosT2", [DR, SEQ], F32R, kind="ExternalInput").ap()
    sinT2 = nc.dram_tensor("sinT2", [DR, SEQ], F32R, kind="ExternalInput").ap()
    ones_d = nc.dram_tensor("ones", [128, 1], F32R, kind="ExternalInput").ap()
    emask_d = nc.dram_tensor("emask", [KP, NM * QP], F32R,
                             kind="ExternalInput").ap() \
        .rearrange("p (b n) -> p b n", b=NM)
    lam_d = nc.dram_tensor("lam", [1, 1], F32, kind="ExternalInput").ap()
    yT = nc.dram_tensor("yT", [DIM, SEQ], F32, kind="ExternalOutput").ap()

    with tile.TileContext(nc) as tc:
      with tc.tile_pool(name="shared", bufs=1) as pp:
        onest = pp.tile([128, 1], F32R, tag="ones")
        nc.sync.dma_start(out=onest, in_=ones_d)
        lamt = pp.tile([1, 1], F32, tag="lam")
        nc.sync.dma_start(out=lamt, in_=lam_d)
        cost = pp.tile([DR, SEQ], F32R, tag="cos")
        nc.sync.dma_start(out=cost, in_=cosT2)
        sint = pp.tile([DR, SEQ], F32R, tag="sin")
        nc.sync.dma_start(out=sint, in_=sinT2)
        emt = pp.tile([KP, NM, QP], F32R, tag="em")
        if n_emask:
            nc.sync.dma_start(out=emt, in_=emask_d)

        for _rep in range(repeat):
            # ======== persistent head tensors (B..E) ========
            with tc.tile_pool(name="heads", bufs=1) as hp:
                qnope = [hp.tile([128, SEQ], F32R, name=f"qn{h}", tag=f"qn{h}")
                         for h in range(HPC)]
                knope = [hp.tile([128, SEQ], F32R, name=f"kn{h}", tag=f"kn{h}")
                         for h in range(HPC)]
                vv = [hp.tile([128, NKB, DV], F32R, name=f"v{h}", tag=f"v{h}")
                      for h in range(HPC)]
                outT = [hp.tile([128, SEQ], F32R, name=f"o{h}", tag=f"o{h}")
                        for h in range(HPC)]
                qrope = hp.tile([128, SEQ], F32R, tag="qrope")
                kpe = hp.tile([DR, SEQ], F32R, tag="kpe")
                nc.sync.dma_start(out=kpe, in_=kpe_d)

                # ======== phase B: head projections ========
                with tc.tile_pool(name="phb", bufs=1) as pb, \
                     tc.tile_pool(name="phbs", bufs=2) as pbs, \
                     tc.tile_pool(name="psB", bufs=4, space="PSUM") as psB, \
                     tc.tile_pool(name="psV", bufs=4, space="PSUM") as psV:
                    wqb = pb.tile([128, KQ, HPC * QKH], F32R, tag="wqb")
                    nc.sync.dma_start(out=wqb, in_=wq_bT)
                    wkvb = pb.tile([128, KV, HPC * (DN + DV)], F32R, tag="wkvb")
                    nc.sync.dma_start(out=wkvb, in_=wkv_bT)

                    for n in range(NQB):
                        nsl = slice(n * QP, (n + 1) * QP)
                        qa_n = pbs.tile([128, KQ, QP], F32R, tag="qan")
                        nc.sync.dma_start(out=qa_n, in_=qa_d[:, :, nsl])
                        kv_n = pbs.tile([128, KV, QP], F32R, tag="kvn")
                        nc.sync.dma_start(out=kv_n, in_=kv_d[:, :, nsl])

                        for mi, dest in enumerate(qnope + [qrope]):
                            ps = psB.tile([128, QP], F32, tag="psB")
                            for k in range(KQ):
                                nc.tensor.matmul(
                                    ps, wqb[:, k, mi * 128:(mi + 1) * 128],
                                    qa_n[:, k, :],
                                    start=(k == 0), stop=(k == KQ - 1))
                            nc.vector.tensor_copy(dest[:, nsl], ps)
                        for h in range(HPC):
                            co = h * (DN + DV)
                            ps = psB.tile([128, QP], F32, tag="psB")
                            for k in range(KV):
                                nc.tensor.matmul(
                                    ps, wkvb[:, k, co:co + DN], kv_n[:, k, :],
                                    start=(k == 0), stop=(k == KV - 1))
                            nc.vector.tensor_copy(knope[h][:, nsl], ps)
                            for smi in range(QP // 128):
                                sm = n * (QP // 128) + smi
                                psv = psV.tile([128, DV], F32, tag="psV")
                                for k in range(KV):
                                    nc.tensor.matmul(
                                        psv,
                                        kv_n[:, k, smi * 128:(smi + 1) * 128],
                                        wkvb[:, k, co + DN:co + DN + DV],
                                        start=(k == 0), stop=(k == KV - 1))
                                nc.vector.tensor_copy(vv[h][:, sm, :], psv)

                if until <= 2:
                    continue
                # ======== phases C+D scoped tensors ========
                with tc.tile_pool(name="cd", bufs=1) as cd:
                    q2 = [cd.tile([H, SEQ], F32R, name=f"q2{h}", tag=f"q2{h}")
                          for h in range(HPC)]
                    k2 = [cd.tile([H, SEQ], F32R, name=f"k2{h}", tag=f"k2{h}")
                          for h in range(HPC)]

                    # ---- phase C: rope (DVE at partition base 0 only;
                    #      partition moves via SBUF->SBUF DMA) ----
                    with tc.tile_pool(name="phc", bufs=1) as pc:
                        qxi = pc.tile([DR, SEQ], F32R, tag="qxi")
                        nc.sync.dma_start(out=qxi, in_=qrope[64:128])
                        qxr = qrope[0:64]
                        cos64, sin64 = cost[0:64], sint[0:64]
                        yr64 = pc.tile([DR, SEQ], F32R, tag="yr64")
                        yi64 = pc.tile([DR, SEQ], F32R, tag="yi64")
                        ta = pc.tile([DR, SEQ], F32R, tag="ropetmp")
                        tb = pc.tile([DR, SEQ], F32R, tag="ropetmp2")
                        nc.vector.tensor_tensor(ta, qxr, cos64, MM.mult)
                        nc.vector.tensor_tensor(tb, qxi, sin64, MM.mult)
                        nc.vector.tensor_tensor(yr64, ta, tb, MM.subtract)
                        ta = pc.tile([DR, SEQ], F32R, tag="ropetmp")
                        tb = pc.tile([DR, SEQ], F32R, tag="ropetmp2")
                        nc.vector.tensor_tensor(ta, qxr, sin64, MM.mult)
                        nc.vector.tensor_tensor(tb, qxi, cos64, MM.mult)
                        nc.vector.tensor_tensor(yi64, ta, tb, MM.add)

                        kpi = pc.tile([32, SEQ], F32R, tag="kpi")
                        nc.sync.dma_start(out=kpi, in_=kpe[32:64])
                        kpr = kpe[0:32]
                        cos32, sin32 = cost[0:32], sint[0:32]
                        kr32 = pc.tile([32, SEQ], F32R, tag="kr32")
                        ki32 = pc.tile([32, SEQ], F32R, tag="ki32")
                        ta = pc.tile([32, SEQ], F32R, tag="ropetmp")
                        tb = pc.tile([32, SEQ], F32R, tag="ropetmp2")
                        nc.vector.tensor_tensor(ta, kpr, cos32, MM.mult)
                        nc.vector.tensor_tensor(tb, kpi, sin32, MM.mult)
                        nc.vector.tensor_tensor(kr32, ta, tb, MM.subtract)
                        ta = pc.tile([32, SEQ], F32R, tag="ropetmp")
                        tb = pc.tile([32, SEQ], F32R, tag="ropetmp2")
                        nc.vector.tensor_tensor(ta, kpr, sin32, MM.mult)
                        nc.vector.tensor_tensor(tb, kpi, cos32, MM.mult)
                        nc.vector.tensor_tensor(ki32, ta, tb, MM.add)

                        for h in range(HPC):
                            nc.sync.dma_start(out=q2[h][0:32],
                                              in_=qnope[h][96:128])
                            nc.sync.dma_start(out=q2[h][32:64],
                                              in_=yr64[h * 32:(h + 1) * 32])
                            nc.sync.dma_start(out=q2[h][64:96],
                                              in_=yi64[h * 32:(h + 1) * 32])
                            nc.sync.dma_start(out=k2[h][0:32],
                                              in_=knope[h][96:128])
                            nc.sync.dma_start(out=k2[h][32:64], in_=kr32)
                            nc.sync.dma_start(out=k2[h][64:96], in_=ki32)

                    # ---- phase D: differential attention, k-major ----
                    with tc.tile_pool(name="phd", bufs=3) as pd, \
                         tc.tile_pool(name="phd2", bufs=1) as pd2, \
                         tc.tile_pool(name="po", bufs=1, space="PSUM") as poolo, \
                         tc.tile_pool(name="pz", bufs=1, space="PSUM") as poolz, \
                         tc.tile_pool(name="pscr", bufs=2, space="PSUM") as pscr:
                        for h in range(HPC):
                            for n in range(NQB):
                                m_list = [m for m in range(NKB)
                                          if block_kind[m][n] != 1]
                                qsl = slice(n * QP, (n + 1) * QP)
                                o1 = poolo.tile([128, QP], F32, tag="o1")
                                o2 = poolo.tile([128, QP], F32, tag="o2")
                                z1 = poolz.tile([1, QP], F32, tag="z1")
                                z2 = poolz.tile([1, QP], F32, tag="z2")
                                for i, m in enumerate(m_list):
                                    ksl = slice(m * KP, (m + 1) * KP)
                                    first = (i == 0)
                                    last = (i == len(m_list) - 1)
                                    s1 = pscr.tile([KP, QP], F32, tag="s1")
                                    s2 = pscr.tile([KP, QP], F32, tag="s2")
                                    nc.tensor.matmul(s1, knope[h][0:H, ksl],
                                                     qnope[h][0:H, qsl],
                                                     start=True, stop=True)
                                    nc.tensor.matmul(s2, k2[h][:, ksl],
                                                     q2[h][:, qsl],
                                                     start=True, stop=True)
                                    e1 = pd.tile([KP, QP], F32R, tag="e1")
                                    e2 = pd.tile([KP, QP], F32R, tag="e2")
                                    if block_kind[m][n] == 2:
                                        bm = int(block_maskid[m][n])
                                        et1 = pd.tile([KP, QP], F32R, tag="et1")
                                        et2 = pd.tile([KP, QP], F32R, tag="et2")
                                        nc.scalar.activation(et1, s1, Exp)
                                        nc.scalar.activation(et2, s2, Exp)
                                        nc.vector.tensor_tensor(
                                            e1, et1, emt[:, bm, :], MM.mult)
                                        nc.vector.tensor_tensor(
                                            e2, et2, emt[:, bm, :], MM.mult)
                                    else:
                                        nc.scalar.activation(e1, s1, Exp)
                                        nc.scalar.activation(e2, s2, Exp)
                                    nc.tensor.matmul(z1, onest, e1,
                                                     start=first, stop=last)
                                    nc.tensor.matmul(z2, onest, e2,
                                                     start=first, stop=last)
                                    nc.tensor.matmul(o1, vv[h][:, m, :], e1,
                                                     start=first, stop=last)
                                    nc.tensor.matmul(o2, vv[h][:, m, :], e2,
                                                     start=first, stop=last)
                                # normalize: outT = o1/z1 - lam * o2/z2
                                tz1 = pd2.tile([1, QP], F32, tag="tz1")
                                nc.vector.tensor_copy(tz1, z1)
                                tr1 = pd2.tile([1, QP], F32, tag="tr1")
                                nc.vector.reciprocal(tr1, tz1)
                                tz2 = pd2.tile([1, QP], F32, tag="tz2")
                                nc.vector.tensor_copy(tz2, z2)
                                tr2 = pd2.tile([1, QP], F32, tag="tr2")
                                nc.vector.reciprocal(tr2, tz2)
                                tr2l = pd2.tile([1, QP], F32, tag="tr2l")
                                nc.vector.tensor_scalar(tr2l, tr2, lamt, None,
                                                        MM.mult)
                                rb1 = pd2.tile([128, QP], F32, tag="rb1")
                                nc.gpsimd.partition_broadcast(rb1, tr1)
                                rb2 = pd2.tile([128, QP], F32, tag="rb2")
                                nc.gpsimd.partition_broadcast(rb2, tr2l)
                                t1 = pd2.tile([128, QP], F32, tag="t1")
                                nc.vector.tensor_tensor(t1, o1, rb1, MM.mult)
                                t2 = pd2.tile([128, QP], F32, tag="t2")
                                nc.vector.tensor_tensor(t2, o2, rb2, MM.mult)
                                nc.vector.tensor_tensor(outT[h][:, qsl], t1, t2,
                                                        MM.subtract)

                if until <= 3:
                    continue
                # ======== phase E: output projection (partial y) ========
                with tc.tile_pool(name="phe", bufs=1) as pe, \
                     tc.tile_pool(name="phey", bufs=4) as pey, \
                     tc.tile_pool(name="psE", bufs=4, space="PSUM") as psE:
                    wot = pe.tile([128, 2, DIM], F32R, tag="wot")
                    nc.sync.dma_start(out=wot, in_=woT)
                    for mo in range(DIM // 128):
                        for n in range(NQB):
                            ps = psE.tile([128, QP], F32, tag="psE")
                            for k in range(HPC):
                                nc.tensor.matmul(
                                    ps, wot[:, k, mo * 128:(mo + 1) * 128],
                                    outT[k][:, n * QP:(n + 1) * QP],
                                    start=(k == 0), stop=(k == HPC - 1))
                            ty = pey.tile([128, QP], F32, tag="ty")
                            nc.vector.tensor_copy(ty, ps)
                            nc.sync.dma_start(
                                out=yT[mo * 128:(mo + 1) * 128,
                                       n * QP:(n + 1) * QP], in_=ty)


def _build_nc(inputs, repeat=1):
    """Build both launch programs; returns (nc_a, nc_b, in_maps_a, in_maps_b)."""
    in_maps_a, in_maps_b, block_kind, block_maskid, n_emask = _host_prep(inputs)
    nc_a = bacc.Bacc("TRN2", target_bir_lowering=False, debug=False,
                     num_devices=N_CORES)
    _build_a(nc_a, repeat=repeat)
    nc_a.compile()
    nc_b = bacc.Bacc("TRN2", target_bir_lowering=False, debug=False,
                     num_devices=N_CORES)
    _build_be(nc_b, block_kind, block_maskid, n_emask, repeat=repeat)
    nc_b.compile()
    return nc_a, nc_b, in_maps_a, in_maps_b


def _gather_a(results_a):
    """Host gather of launch-1 outputs into full qa/kv/kpe arrays."""
    KQ = QLR // 128
    KV = KVLR // 128
    qa = np.empty((128, KQ, SEQ), np.float32)
    kv = np.empty((128, KV, SEQ), np.float32)
    kpe = np.empty((DR, SEQ), np.float32)
    for c, r in enumerate(results_a):
        sl = slice(c * SW, (c + 1) * SW)
        qa[:, :, sl] = r["qa_s"].reshape(128, KQ, SW)
        kv[:, :, sl] = r["kv_s"].reshape(128, KV, SW)
        kpe[:, sl] = r["kpe_s"]
    return (qa.reshape(128, -1), kv.reshape(128, -1),
            np.ascontiguousarray(kpe))


def kernel(**inputs):
    inputs = {k: np.asarray(v) for k, v in inputs.items()}
    nc_a, nc_b, in_maps_a, in_maps_b = _build_nc(inputs)
    res_a = run_bass_kernel_spmd(nc_a, in_maps_a,
                                 core_ids=list(range(N_CORES)))
    qa, kv, kpe = _gather_a(res_a.results)
    for m in in_maps_b:
        m["qa_d"] = qa
        m["kv_d"] = kv
        m["kpe_d"] = kpe
    res_b = run_bass_kernel_spmd(nc_b, in_maps_b,
                                 core_ids=list(range(N_CORES)))
    yT_sum = np.zeros((DIM, SEQ), np.float64)
    for r in res_b.results:
        yT_sum += r["yT"]
    return np.ascontiguousarray(yT_sum.T).reshape(1, SEQ, DIM).astype(np.float32)

